# revision 27
# baseline (speedup 1.0000x reference)
"""Trainium2 Bass kernel for a dense pre-norm transformer block.

B, S, H, NH, MLP = 4, 2048, 768, 12, 3072 (fp32 I/O).

Sharding: 8 shards = (batch, seq-half). Each core receives its batch's full
2048-token sequence with its own 1024 query tokens permuted to the front
(attention is permutation-invariant over keys), computes K/V for all 2048
tokens, and Q/attention/MLP for its 1024 query tokens. No collectives.

On-chip: activations are kept feature-major [feature-part, token-free] for
matmuls (weights stationary), token-major for LN/softmax-normalize/residual.
Attention computes scoresT = K @ Q^T per head, exponentiates on ACT
(scale=1/8 folded), then multiplies with a stationary [V | ones] so the
softmax denominator accumulates for free in the extra PSUM row; the
normalization happens after a PE transpose back to token-major where the
denominator is a per-partition scalar.

fp8 (e4m3) + DoubleRow perf mode doubles matmul throughput for every
contraction>=256 GEMM: Q/K/V/O projections, the probs@V context matmul
(kt-pair slabs), and both MLP linears. Weights are pre-scaled by WS on the
host so their values sit in e4m3's normal range; the dequant (1/WS) rides
the PSUM-eviction op (fused with bias). Scores stay bf16 (contraction=64
gains nothing from fp8). fp32 accumulation everywhere; fp32 LN/residual.

Schedule: the Q/K projections are interleaved with attention per head-pair so
the tensor engine never idles waiting on ACT exp; PSUM->SBUF copies ride on
DVE/ACT to balance engine load.
"""

import sys

if "/opt/trn_rl_repo" not in sys.path:
    sys.path.insert(0, "/opt/trn_rl_repo")

from contextlib import ExitStack

import ml_dtypes
import numpy as np

import concourse.bacc as bacc
import concourse.bass as bass
import concourse.mybir as mybir
import concourse.tile as tile
from concourse.alu_op_type import AluOpType
from concourse.bass_utils import run_bass_kernel_spmd
from concourse.masks import make_identity

B, S, H, NH, MLPD = 4, 2048, 768, 12, 3072
HD = H // NH  # 64
EPS = 1e-6
P = 128
N_H = H // P  # 6
N_M = MLPD // P  # 24
AF = mybir.ActivationFunctionType
BF = mybir.dt.bfloat16
F32 = mybir.dt.float32
F8 = mybir.dt.float8e4
DR = mybir.MatmulPerfMode.DoubleRow

# fp8 toggles + scales
FP8_ATTN = True
FP8_FC1 = True
FP8_FC2 = False  # fc2 stays bf16: fp8 on both MLP linears costs ~1.8e-2 rel
WS = 1024.0  # host-side fp8 weight pre-scale
DQ = 1.0 / WS
CTX_S = 64.0  # ctx quantization scale (ctx values are small)

_BUILD_CACHE = {}


def build(tkv=S, mlp_act="Gelu"):
    key = (tkv, mlp_act)
    if key in _BUILD_CACHE:
        return _BUILD_CACHE[key]

    tq = tkv // 2
    n_kv = tkv // P  # K/V token tiles
    n_q = tq // P  # query token tiles
    CH = 512 if tq % 512 == 0 else tq  # moving-operand chunk
    n_cq = tq // CH  # query chunks
    n_ckv = tkv // CH  # kv chunks
    n_b = CH // P  # 128-blocks per chunk
    VC = 384  # v-proj output chunk (6 heads)
    n_vc = H // VC  # 2

    WQ_T = F8 if FP8_ATTN else BF
    W1_T = F8 if FP8_FC1 else BF
    W2_T = F8 if FP8_FC2 else BF
    XN_T = F8 if FP8_ATTN else BF
    XN2_T = F8 if FP8_FC1 else BF
    H1_T = F8 if FP8_FC2 else BF
    EX_T = F8 if FP8_ATTN else BF
    CTX_T = F8 if FP8_ATTN else BF
    ADQ = DQ if FP8_ATTN else 1.0
    CSC = CTX_S if FP8_ATTN else 1.0

    nc = bacc.Bacc("TRN2", target_bir_lowering=False, debug=False, num_devices=8)

    x_d = nc.dram_tensor("x_loc", (tkv, H), F32, kind="ExternalInput").ap()
    wq_d = nc.dram_tensor("wq", (H, H), WQ_T, kind="ExternalInput").ap()
    wk_d = nc.dram_tensor("wk", (H, H), WQ_T, kind="ExternalInput").ap()
    wv_d = nc.dram_tensor("wv", (H, H), WQ_T, kind="ExternalInput").ap()
    wo_d = nc.dram_tensor("wo", (H, H), WQ_T, kind="ExternalInput").ap()
    w1_d = nc.dram_tensor("w1", (H, MLPD), W1_T, kind="ExternalInput").ap()
    w2_d = nc.dram_tensor("w2", (MLPD, H), W2_T, kind="ExternalInput").ap()
    bq_d = nc.dram_tensor("bq", (H,), F32, kind="ExternalInput").ap()
    bk_d = nc.dram_tensor("bk", (H,), F32, kind="ExternalInput").ap()
    bv_d = nc.dram_tensor("bv", (H,), BF, kind="ExternalInput").ap()
    bo_d = nc.dram_tensor("bo", (H,), F32, kind="ExternalInput").ap()
    b1_d = nc.dram_tensor("b1", (MLPD,), F32, kind="ExternalInput").ap()
    b2_d = nc.dram_tensor("b2", (H,), F32, kind="ExternalInput").ap()
    ln1w_d = nc.dram_tensor("ln1_w", (H,), BF, kind="ExternalInput").ap()
    ln1b_d = nc.dram_tensor("ln1_b", (H,), BF, kind="ExternalInput").ap()
    ln2w_d = nc.dram_tensor("ln2_w", (H,), BF, kind="ExternalInput").ap()
    ln2b_d = nc.dram_tensor("ln2_b", (H,), BF, kind="ExternalInput").ap()
    out_d = nc.dram_tensor("out_loc", (tq, H), F32, kind="ExternalOutput").ap()

    def bcast(ap1d):
        return bass.AP(
            tensor=ap1d.tensor, offset=ap1d.offset, ap=[[0, P]] + list(ap1d.ap)
        )

    with tile.TileContext(nc) as tc, ExitStack() as top:
        const = top.enter_context(tc.tile_pool(name="const", bufs=1))
        persist = top.enter_context(tc.tile_pool(name="persist", bufs=1))
        # Top-level PSUM pool: 2 banks shared by transposes + proj accums.
        psum = top.enter_context(tc.tile_pool(name="psum", bufs=1, space="PSUM"))
        toks = top.enter_context(tc.tile_pool(name="toks", bufs=4))
        tmps = top.enter_context(tc.tile_pool(name="tmps", bufs=2))

        # ---- constants ----
        ident = const.tile([P, P], BF)
        make_identity(nc, ident)
        eps_t = const.tile([P, 1], F32)
        nc.vector.memset(eps_t, EPS)
        ln1w_bc = const.tile([P, H], BF)
        nc.gpsimd.dma_start(out=ln1w_bc, in_=bcast(ln1w_d))
        ln1b_bc = const.tile([P, H], BF)
        nc.gpsimd.dma_start(out=ln1b_bc, in_=bcast(ln1b_d))
        ln2w_bc = const.tile([P, H], BF)
        nc.gpsimd.dma_start(out=ln2w_bc, in_=bcast(ln2w_d))
        ln2b_bc = const.tile([P, H], BF)
        nc.gpsimd.dma_start(out=ln2b_bc, in_=bcast(ln2b_d))
        bv_row = const.tile([1, H], BF)
        nc.sync.dma_start(out=bv_row, in_=bv_d[None, :])
        ones_row = const.tile([1, P], BF)
        nc.vector.memset(ones_row, 1.0)
        bq_sb = const.tile([P, N_H], F32)
        nc.sync.dma_start(out=bq_sb, in_=bq_d.rearrange("(t p) -> p t", p=P))
        bk_sb = const.tile([P, N_H], F32)
        nc.sync.dma_start(out=bk_sb, in_=bk_d.rearrange("(t p) -> p t", p=P))
        bo_sb = const.tile([P, N_H], F32)
        nc.sync.dma_start(out=bo_sb, in_=bo_d.rearrange("(t p) -> p t", p=P))
        b1_sb = const.tile([P, N_M], F32)
        nc.sync.dma_start(out=b1_sb, in_=b1_d.rearrange("(t p) -> p t", p=P))
        b2_sb = const.tile([P, N_H], F32)
        nc.sync.dma_start(out=b2_sb, in_=b2_d.rearrange("(t p) -> p t", p=P))
        # Weight layout [P, pair, out-tile, slab, P]: the DoubleRow stationary
        # slice [:, g, hot, :, :] is a contiguous 2x128 block (the ISA
        # requires the two slabs adjacent in SBUF).
        wo_sb = const.tile([P, N_H // 2, N_H, 2, P], WQ_T)

        def load_w_pairs(dst, src_d, n_in, n_out):
            for g in range(n_in // 2):
                for s in range(2):
                    i = 2 * g + s
                    nc.sync.dma_start(
                        out=dst[:, g, :, s, :], in_=src_d[i * P : (i + 1) * P, :]
                    )

        ctx_tok = persist.tile([P, n_q, H], BF)  # normalized ctx (token-major)

        def ln_tile(x_ap, w_bc, b_bc, out_bf):
            """LayerNorm of one [P, H] fp32 tile -> bf16 out (token-major).

            Stats on DVE, the normalize pass on ACT (per-partition
            scale/bias), the weight/bias application on DVE in bf16.
            """
            stats = tmps.tile([P, 2, 6], F32, tag="ln_stats", bufs=4)
            for g in range(2):
                nc.vector.bn_stats(out=stats[:, g, :], in_=x_ap[:, g * 384 : (g + 1) * 384])
            mv = tmps.tile([P, 2], F32, tag="ln_mv", bufs=4)
            nc.vector.bn_aggr(out=mv, in_=stats)
            rstd = tmps.tile([P, 1], F32, tag="ln_rstd", bufs=4)
            nc.scalar.activation(out=rstd, in_=mv[:, 1:2], func=AF.Sqrt, bias=eps_t, scale=1.0)
            nc.vector.reciprocal(out=rstd, in_=rstd)
            nmr = tmps.tile([P, 1], F32, tag="ln_nmr", bufs=4)
            nc.vector.scalar_tensor_tensor(
                out=nmr, in0=mv[:, 0:1], scalar=-1.0, in1=rstd,
                op0=AluOpType.mult, op1=AluOpType.mult,
            )
            xh = tmps.tile([P, H], BF, tag="ln_xh", bufs=4)
            nc.scalar.activation(out=xh, in_=x_ap, func=AF.Identity, scale=rstd, bias=nmr)
            nc.vector.tensor_mul(out_bf, xh, w_bc)
            nc.vector.tensor_add(out_bf, out_bf, b_bc)

        def transpose_to(dst_ap, src_ap, rows, cols):
            """dst[cols, rows] = src[rows, cols].T (SBUF->SBUF, evict on DVE).

            The PE transpose stays bf16 (fp8 transpose needs stride-2
            outputs); any fp8 cast happens in the DVE eviction.
            """
            pt = psum.tile([P, P], BF, tag="aux", bufs=2)
            nc.tensor.transpose(pt[0:cols, 0:rows], src_ap, ident[0:rows, 0:rows])
            nc.vector.tensor_copy(out=dst_ap, in_=pt[0:cols, 0:rows])

        # ====== Phase 1-3: LN1, V proj, then attention in two query
        # chunks. Chunk 0 runs with the Q/K projections interleaved; chunk 1
        # runs with chunk 0's downstream work (ctx transpose, out-proj,
        # residual, LN2) woven between head-pairs so the PE chews projection
        # work while ACT grinds exp. ======
        ACH = CH  # attention query-chunk = moving-operand chunk (512)
        n_ac = tq // ACH

        late = top.enter_context(tc.tile_pool(name="late", bufs=1))
        x1_sb = late.tile([P, n_q, H], F32)  # attn-block out (token-major)
        xn2T = late.tile([P, N_H, tq], XN2_T)
        op = top.enter_context(tc.tile_pool(name="oproj", bufs=1))

        with tc.tile_pool(name="qkv_sb", bufs=1) as qkv_sb:
            # Q stored zero-padded per head: head h occupies its 64 rows,
            # the other 64 rows stay zero, so the scores matmul can use the
            # full 128-row kT stationary (FWL) with exact math.
            qT = qkv_sb.tile([P, NH, tq], BF)
            nc.vector.memset(qT, 0.0)
            kT = qkv_sb.tile([P, N_H, tkv], BF)
            # [P, kt-pair, head, slab, 128]: ctx DoubleRow stationary slice
            # [:, ktp, h, :, 0:65] — the ISA wants slab stride 128, so the
            # last dim is padded from HD+1 to 128.
            vone = qkv_sb.tile([P, n_kv // 2, NH, 2, P], EX_T)
            nc.vector.memset(vone[:, :, :, :, HD : HD + 1], 1.0)

            with tc.tile_pool(name="ln_qkv", bufs=1) as lnp, tc.tile_pool(
                name="attn_sb", bufs=1
            ) as asb:
                xnT = lnp.tile([P, N_H, tkv], XN_T)
                wq_sb = lnp.tile([P, N_H // 2, N_H, 2, P], WQ_T)
                wk_sb = lnp.tile([P, N_H // 2, N_H, 2, P], WQ_T)
                wv_sb = lnp.tile([P, N_H, H], WQ_T)
                for i in range(N_H):
                    nc.sync.dma_start(
                        out=wv_sb[:, i, :], in_=wv_d[i * P : (i + 1) * P, :]
                    )

                # LN1 + transpose + V projection, per token tile (keeps PE
                # fed with V matmuls while DVE/ACT chew the next LN). The
                # attention PSUM pool opens only after this loop, so V
                # accumulators and transposes get their own banks here.
                with tc.tile_pool(name="psLN", bufs=1, space="PSUM") as psLN:
                    for t in range(n_kv):
                        x_t = toks.tile([P, H], F32, tag="xtok")
                        nc.sync.dma_start(out=x_t, in_=x_d[t * P : (t + 1) * P, :])
                        xn_bf = tmps.tile([P, H], BF, tag="xn_bf", bufs=4)
                        ln_tile(x_t, ln1w_bc, ln1b_bc, xn_bf)
                        for j in range(N_H):
                            transpose_to(
                                xnT[:, j, t * P : (t + 1) * P],
                                xn_bf[:, j * P : (j + 1) * P], P, P,
                            )
                        for c2 in range(n_vc):
                            pv = psLN.tile([P, VC], F32, tag="pv", bufs=3)
                            # bias row via K=1 ones-matmul, then accumulate
                            nc.tensor.matmul(
                                pv, ones_row[:, 0:P],
                                bv_row[:, c2 * VC : (c2 + 1) * VC],
                                start=True, stop=False,
                            )
                            # V-proj stays non-DoubleRow: its stationary is
                            # the activation tile, which can't be pair-packed
                            # without breaking the QK moving layout.
                            for hit in range(N_H):
                                nc.tensor.matmul(
                                    pv,
                                    xnT[:, hit, t * P : (t + 1) * P],
                                    wv_sb[:, hit, c2 * VC : (c2 + 1) * VC],
                                    start=False, stop=(hit == N_H - 1),
                                )
                            vdst = vone[
                                :, t // 2, c2 * (VC // HD) : (c2 + 1) * (VC // HD),
                                t % 2, 0:HD,
                            ]
                            if FP8_ATTN:
                                nc.scalar.mul(
                                    vdst, pv.rearrange("p (h d) -> p h d", d=HD), DQ,
                                )
                            else:
                                nc.vector.tensor_copy(
                                    out=vdst,
                                    in_=pv.rearrange("p (h d) -> p h d", d=HD),
                                )

                # Q/K/O weights only needed once attention starts; emit
                # their loads after the x/LN traffic so they don't delay it.
                load_w_pairs(wq_sb, wq_d, N_H, N_H)
                load_w_pairs(wk_sb, wk_d, N_H, N_H)
                load_w_pairs(wo_sb, wo_d, N_H, N_H)

                def qk_proj_chunk(w_sb, b_sb, dstT, hot, c, split_q=False):
                        pk = psum.tile([P, CH], F32, tag="aux", bufs=2)
                        if FP8_ATTN:
                            for g in range(N_H // 2):
                                nc.tensor.matmul(
                                    pk,
                                    w_sb[:, g, hot, :, :],
                                    xnT[:, 2 * g : 2 * g + 2, c * CH : (c + 1) * CH],
                                    start=(g == 0), stop=(g == N_H // 2 - 1),
                                    perf_mode=DR,
                                )
                        else:
                            for hit in range(N_H):
                                nc.tensor.matmul(
                                    pk,
                                    w_sb[:, hit // 2, hot, hit % 2, :],
                                    xnT[:, hit, c * CH : (c + 1) * CH],
                                    start=(hit == 0), stop=(hit == N_H - 1),
                                )
                        if split_q:
                            nc.vector.tensor_scalar(
                                out=dstT[0:HD, 2 * hot, c * CH : (c + 1) * CH],
                                in0=pk[0:HD, :], scalar1=ADQ,
                                scalar2=b_sb[:, hot : hot + 1][0:HD],
                                op0=AluOpType.mult, op1=AluOpType.add,
                            )
                            nc.vector.tensor_scalar(
                                out=dstT[HD:P, 2 * hot + 1, c * CH : (c + 1) * CH],
                                in0=pk[HD:P, :], scalar1=ADQ,
                                scalar2=b_sb[:, hot : hot + 1][HD:P],
                                op0=AluOpType.mult, op1=AluOpType.add,
                            )
                        else:
                            nc.vector.tensor_scalar(
                                out=dstT[:, hot, c * CH : (c + 1) * CH],
                                in0=pk, scalar1=ADQ,
                                scalar2=b_sb[:, hot : hot + 1],
                                op0=AluOpType.mult, op1=AluOpType.add,
                            )

                def qk_thunks(ht):
                    ths = []
                    for c in range(n_cq):
                        ths.append(lambda c=c: qk_proj_chunk(
                            wq_sb, bq_sb, qT, ht, c, split_q=True))
                    for c in range(n_ckv):
                        ths.append(lambda c=c: qk_proj_chunk(
                            wk_sb, bk_sb, kT, ht, c))
                    return ths

                psA = []

                def attention_head(h, c, fill=None):
                    ht = h // 2
                    pctx = psA[0].tile([P, ACH], F32, tag="pctx", bufs=2)
                    for ktp in range(n_kv // 2):
                        if fill is not None:
                            for th in (next(fill, None),):
                                if th is not None:
                                    th()
                        # kt-pair scores land in one [P, 2, ACH] PSUM tile so
                        # a single 1024-col exp serves both (ACT has ~100ns
                        # fixed cost per instruction).
                        ps = psA[0].tile([P, 2, ACH], F32, tag="psc", bufs=2)
                        for s in range(2):
                            kt = 2 * ktp + s
                            nc.tensor.matmul(
                                ps[:, s, :],
                                kT[:, ht, kt * P : (kt + 1) * P],
                                qT[:, h, c * ACH : (c + 1) * ACH],
                                start=True, stop=True,
                            )
                        ex2 = asb.tile([P, 2, ACH], EX_T, tag="exp", bufs=4)
                        nc.scalar.activation(
                            out=ex2, in_=ps, func=AF.Exp, scale=0.125,
                        )
                        if FP8_ATTN:
                            nc.tensor.matmul(
                                pctx[0 : HD + 1, :],
                                vone[:, ktp, h, :, 0 : HD + 1],
                                ex2,
                                start=(ktp == 0), stop=(ktp == n_kv // 2 - 1),
                                perf_mode=DR,
                            )
                        else:
                            for s in range(2):
                                nc.tensor.matmul(
                                    pctx[0 : HD + 1, :],
                                    vone[:, ktp, h, s, 0 : HD + 1],
                                    ex2[:, s, :],
                                    start=(ktp == 0 and s == 0),
                                    stop=(ktp == n_kv // 2 - 1 and s == 1),
                                )
                    cd = asb.tile([P, ACH], BF, tag="cd", bufs=3)
                    nc.vector.tensor_copy(out=cd[0 : HD + 1, :], in_=pctx[0 : HD + 1, :])
                    for b4 in range(ACH // P):
                        t_tok = c * (ACH // P) + b4
                        pt = psum.tile([P, P], BF, tag="aux", bufs=2)
                        nc.tensor.transpose(
                            pt[0:P, 0 : HD + 1],
                            cd[0 : HD + 1, b4 * P : (b4 + 1) * P],
                            ident[0 : HD + 1, 0 : HD + 1],
                        )
                        rp = tmps.tile([P, 1], F32, tag="rp", bufs=4)
                        nc.vector.reciprocal(rp, pt[:, HD : HD + 1])
                        nc.vector.tensor_scalar(
                            out=ctx_tok[:, t_tok, h * HD : (h + 1) * HD],
                            in0=pt[:, 0:HD], scalar1=rp, scalar2=CSC,
                            op0=AluOpType.mult, op1=AluOpType.mult,
                        )

                # ---- downstream for one query chunk: ctx transpose,
                # out-proj, residual, LN2 (+ xn2 transpose). Returned as a
                # list of thunks so chunk 0's work can be interleaved
                # between chunk 1's attention head-pairs. ----
                def downstream_pieces(c):
                    ctxT = op.tile([P, N_H, CH], CTX_T, tag="ctxT", bufs=2)
                    uT = op.tile([P, N_H, CH], BF, tag="uT", bufs=2)

                    def ctx_trans(ts_):
                        for t in ts_:
                            for j in range(N_H):
                                transpose_to(
                                    ctxT[:, j, (t - c * n_b) * P : (t - c * n_b + 1) * P],
                                    ctx_tok[:, t, j * P : (j + 1) * P], P, P,
                                )

                    def oproj(hots):
                        for hot in hots:
                            pu = psum.tile([P, CH], F32, tag="aux", bufs=2)
                            if FP8_ATTN:
                                for g in range(N_H // 2):
                                    nc.tensor.matmul(
                                        pu,
                                        wo_sb[:, g, hot, :, :],
                                        ctxT[:, 2 * g : 2 * g + 2, :],
                                        start=(g == 0), stop=(g == N_H // 2 - 1),
                                        perf_mode=DR,
                                    )
                            else:
                                for hit in range(N_H):
                                    nc.tensor.matmul(
                                        pu,
                                        wo_sb[:, hit // 2, hot, hit % 2, :],
                                        ctxT[:, hit, :],
                                        start=(hit == 0), stop=(hit == N_H - 1),
                                    )
                            nc.vector.tensor_scalar(
                                out=uT[:, hot, :],
                                in0=pu, scalar1=ADQ / CSC,
                                scalar2=bo_sb[:, hot : hot + 1],
                                op0=AluOpType.mult, op1=AluOpType.add,
                            )

                    xrs = {}
                    for t in range(c * n_b, (c + 1) * n_b):
                        xr = toks.tile([P, H], F32, tag="xtok")
                        nc.sync.dma_start(out=xr, in_=x_d[t * P : (t + 1) * P, :])
                        xrs[t] = xr

                    def resid_ln2(ts_):
                        for t in ts_:
                            xr = xrs[t]
                            for j in range(N_H):
                                pt = psum.tile([P, P], BF, tag="aux", bufs=2)
                                nc.tensor.transpose(
                                    pt,
                                    uT[:, j, (t - c * n_b) * P : (t - c * n_b + 1) * P],
                                    ident,
                                )
                                nc.vector.tensor_add(
                                    x1_sb[:, t, j * P : (j + 1) * P],
                                    pt,
                                    xr[:, j * P : (j + 1) * P],
                                )
                            xn2_bf = tmps.tile([P, H], BF, tag="xn_bf", bufs=4)
                            ln_tile(x1_sb[:, t, :], ln2w_bc, ln2b_bc, xn2_bf)
                            for j in range(N_H):
                                transpose_to(
                                    xn2T[:, j, t * P : (t + 1) * P],
                                    xn2_bf[:, j * P : (j + 1) * P], P, P,
                                )

                    b0 = c * n_b
                    return (
                        [lambda t=t: ctx_trans([t]) for t in range(b0, b0 + n_b)]
                        + [lambda h_=h_: oproj([h_]) for h_ in range(N_H)]
                        + [lambda t=t: resid_ln2([t]) for t in range(b0, b0 + n_b)]
                    )

                # chunk 0: attention with the NEXT pair's QK projections
                # dripped into the kt loop so ACT never waits on a PE-only
                # stretch; chunk 1: ditto with chunk 0's downstream pieces.
                with tc.tile_pool(name="psA", bufs=1, space="PSUM") as psA_:
                    psA.append(psA_)
                    for th in qk_thunks(0):
                        th()
                    for ht in range(N_H):
                        fill = iter(qk_thunks(ht + 1) if ht + 1 < N_H else [])
                        attention_head(2 * ht, 0, fill)
                        attention_head(2 * ht + 1, 0, fill)
                    ds0 = iter(downstream_pieces(0))
                    for ht in range(N_H):
                        attention_head(2 * ht, 1, ds0)
                        attention_head(2 * ht + 1, 1, ds0)
                    for th in ds0:
                        th()

        # ========== tail: downstream(c1), then MLP both chunks ==========
        with tc.tile_pool(name="mlp_sb", bufs=1) as mp, tc.tile_pool(
            name="ps6", bufs=1, space="PSUM"
        ) as ps6:
            w1_sb = mp.tile([P, N_H // 2, N_M, 2, P], W1_T)
            w2_sb = mp.tile([P, N_M // 2, N_H, 2, P], W2_T)
            h1c = mp.tile([P, N_M, CH], H1_T)
            y2T = mp.tile([P, N_H, CH], BF)
            # ds(c1) first: its x reloads reach the sync queue ahead of
            # the 7MB of MLP weights; w1 lands under ds(c1), w2 under fc1.
            ds1 = downstream_pieces(1)
            load_w_pairs(w1_sb, w1_d, N_H, N_M)
            for piece in ds1:
                piece()
            load_w_pairs(w2_sb, w2_d, N_M, N_H)

            def mlp_chunk(c):
                for mt in range(N_M):
                    ph = ps6.tile([P, CH], F32, tag="pmm", bufs=4)
                    if FP8_FC1:
                        for g in range(N_H // 2):
                            nc.tensor.matmul(
                                ph,
                                w1_sb[:, g, mt, :, :],
                                xn2T[:, 2 * g : 2 * g + 2, c * CH : (c + 1) * CH],
                                start=(g == 0), stop=(g == N_H // 2 - 1),
                                perf_mode=DR,
                            )
                    else:
                        for hit in range(N_H):
                            nc.tensor.matmul(
                                ph,
                                w1_sb[:, hit // 2, mt, hit % 2, :],
                                xn2T[:, hit, c * CH : (c + 1) * CH],
                                start=(hit == 0), stop=(hit == N_H - 1),
                            )
                    nc.scalar.activation(
                        out=h1c[:, mt, :], in_=ph,
                        func=getattr(AF, mlp_act), bias=b1_sb[:, mt : mt + 1],
                        scale=(DQ if FP8_FC1 else 1.0),
                    )
                for hot in range(N_H):
                    py = ps6.tile([P, CH], F32, tag="pmm", bufs=4)
                    if FP8_FC2:
                        for g in range(N_M // 2):
                            nc.tensor.matmul(
                                py,
                                w2_sb[:, g, hot, :, :],
                                h1c[:, 2 * g : 2 * g + 2, :],
                                start=(g == 0), stop=(g == N_M // 2 - 1),
                                perf_mode=DR,
                            )
                    else:
                        for mt in range(N_M):
                            nc.tensor.matmul(
                                py,
                                w2_sb[:, mt // 2, hot, mt % 2, :],
                                h1c[:, mt, :],
                                start=(mt == 0), stop=(mt == N_M - 1),
                            )
                    nc.vector.tensor_scalar(
                        out=y2T[:, hot, :], in0=py,
                        scalar1=(DQ if FP8_FC2 else 1.0),
                        scalar2=b2_sb[:, hot : hot + 1],
                        op0=AluOpType.mult, op1=AluOpType.add,
                    )
                for b4 in range(n_b):
                    t = c * n_b + b4
                    outt = toks.tile([P, H], F32, tag="xtok")
                    for j in range(N_H):
                        pt = psum.tile([P, P], BF, tag="aux", bufs=2)
                        nc.tensor.transpose(
                            pt, y2T[:, j, b4 * P : (b4 + 1) * P], ident,
                        )
                        nc.vector.tensor_add(
                            outt[:, j * P : (j + 1) * P],
                            pt,
                            x1_sb[:, t, j * P : (j + 1) * P],
                        )
                    nc.sync.dma_start(out=out_d[t * P : (t + 1) * P, :], in_=outt)

            mlp_chunk(0)
            mlp_chunk(1)

    nc.compile()
    _BUILD_CACHE[key] = nc
    return nc


def make_in_maps(inputs, tkv=S):
    """Build the 8 per-core input maps from full inputs."""
    f = np.asarray
    x = f(inputs["x"], dtype=np.float32)
    tq = tkv // 2
    E4NP = ml_dtypes.float8_e4m3

    def wprep(name, fp8):
        w = f(inputs[name], dtype=np.float32)
        if fp8:
            return np.ascontiguousarray((w * WS).astype(E4NP))
        return np.ascontiguousarray(w.astype(ml_dtypes.bfloat16))

    wcast = {
        "wq": wprep("wq", FP8_ATTN),
        "wk": wprep("wk", FP8_ATTN),
        "wv": wprep("wv", FP8_ATTN),
        "wo": wprep("wo", FP8_ATTN),
        "w1": wprep("w1", FP8_FC1),
        "w2": wprep("w2", FP8_FC2),
    }
    fp32v = {
        n: np.ascontiguousarray(f(inputs[n], dtype=np.float32))
        for n in ["bq", "bk", "bo", "b1", "b2"]
    }
    for n in ["ln1_w", "ln1_b", "ln2_w", "ln2_b"]:
        fp32v[n] = np.ascontiguousarray(f(inputs[n]).astype(ml_dtypes.bfloat16))
    bv = f(inputs["bv"], dtype=np.float32)
    if FP8_ATTN:
        bv = bv * WS  # the ones-matmul adds WS*bv into the pre-dequant PSUM
    fp32v["bv"] = np.ascontiguousarray(bv.astype(ml_dtypes.bfloat16))
    in_maps = []
    for c in range(8):
        b, half = c // 2, c % 2
        if half == 0:
            x_loc = x[b, :tkv]
        else:
            x_loc = np.concatenate([x[b, tq:tkv], x[b, :tq]], axis=0)
        m = {"x_loc": np.ascontiguousarray(x_loc)}
        m.update(wcast)
        m.update(fp32v)
        in_maps.append(m)
    return in_maps


def kernel(**inputs):
    nc = build(S)
    in_maps = make_in_maps(inputs, S)
    res = run_bass_kernel_spmd(nc, in_maps, core_ids=list(range(8)))
    tq = S // 2
    out = np.empty((B, S, H), dtype=np.float32)
    for c in range(8):
        b, half = c // 2, c % 2
        out[b, half * tq : (half + 1) * tq] = res.results[c]["out_loc"]
    return out


# revision 28
# speedup vs baseline: 1.1246x; 1.1246x over previous
"""Trainium2 Bass kernel for a dense pre-norm transformer block.

B, S, H, NH, MLP = 4, 2048, 768, 12, 3072 (fp32 I/O).

Sharding: 8 shards = (batch, seq-half). Each core receives its batch's full
2048-token sequence with its own 1024 query tokens permuted to the front
(attention is permutation-invariant over keys), computes K/V for all 2048
tokens, and Q/attention/MLP for its 1024 query tokens. No collectives.

On-chip: activations are kept feature-major [feature-part, token-free] for
matmuls (weights stationary), token-major for LN/softmax-normalize/residual.
Attention computes scoresT = K @ Q^T per head, exponentiates on ACT
(scale=1/8 folded), then multiplies with a stationary [V | ones] so the
softmax denominator accumulates for free in the extra PSUM row; the
normalization happens after a PE transpose back to token-major where the
denominator is a per-partition scalar.

fp8 (e4m3) + DoubleRow perf mode doubles matmul throughput for every
contraction>=256 GEMM: Q/K/V/O projections, the probs@V context matmul
(kt-pair slabs), and both MLP linears. Weights are pre-scaled by WS on the
host so their values sit in e4m3's normal range; the dequant (1/WS) rides
the PSUM-eviction op (fused with bias). Scores stay bf16 (contraction=64
gains nothing from fp8). fp32 accumulation everywhere; fp32 LN/residual.

Schedule: the Q/K projections are interleaved with attention per head-pair so
the tensor engine never idles waiting on ACT exp; PSUM->SBUF copies ride on
DVE/ACT to balance engine load.
"""

import sys

if "/opt/trn_rl_repo" not in sys.path:
    sys.path.insert(0, "/opt/trn_rl_repo")

from contextlib import ExitStack

import ml_dtypes
import numpy as np

import concourse.bacc as bacc
import concourse.bass as bass
import concourse.mybir as mybir
import concourse.tile as tile
from concourse.alu_op_type import AluOpType
from concourse.bass_utils import run_bass_kernel_spmd
from concourse.masks import make_identity

B, S, H, NH, MLPD = 4, 2048, 768, 12, 3072
HD = H // NH  # 64
EPS = 1e-6
P = 128
N_H = H // P  # 6
N_M = MLPD // P  # 24
AF = mybir.ActivationFunctionType
BF = mybir.dt.bfloat16
F32 = mybir.dt.float32
F8 = mybir.dt.float8e4
DR = mybir.MatmulPerfMode.DoubleRow

# fp8 toggles + scales
FP8_ATTN = True
FP8_FC1 = True
FP8_FC2 = False  # fc2 stays bf16: fp8 on both MLP linears costs ~1.8e-2 rel
WS = 1024.0  # host-side fp8 weight pre-scale
DQ = 1.0 / WS
CTX_S = 64.0  # ctx quantization scale (ctx values are small)

_BUILD_CACHE = {}


def build(tkv=S, mlp_act="Gelu"):
    key = (tkv, mlp_act)
    if key in _BUILD_CACHE:
        return _BUILD_CACHE[key]

    tq = tkv // 2
    n_kv = tkv // P  # K/V token tiles
    n_q = tq // P  # query token tiles
    CH = 512 if tq % 512 == 0 else tq  # moving-operand chunk
    n_cq = tq // CH  # query chunks
    n_ckv = tkv // CH  # kv chunks
    n_b = CH // P  # 128-blocks per chunk
    VC = 384  # v-proj output chunk (6 heads)
    n_vc = H // VC  # 2

    WQ_T = F8 if FP8_ATTN else BF
    W1_T = F8 if FP8_FC1 else BF
    W2_T = F8 if FP8_FC2 else BF
    XN_T = F8 if FP8_ATTN else BF
    XN2_T = F8 if FP8_FC1 else BF
    H1_T = F8 if FP8_FC2 else BF
    EX_T = F8 if FP8_ATTN else BF
    CTX_T = F8 if FP8_ATTN else BF
    ADQ = DQ if FP8_ATTN else 1.0
    CSC = CTX_S if FP8_ATTN else 1.0

    nc = bacc.Bacc("TRN2", target_bir_lowering=False, debug=False, num_devices=8)

    x_d = nc.dram_tensor("x_loc", (tkv, H), F32, kind="ExternalInput").ap()
    wq_d = nc.dram_tensor("wq", (H, H), WQ_T, kind="ExternalInput").ap()
    wk_d = nc.dram_tensor("wk", (H, H), WQ_T, kind="ExternalInput").ap()
    wv_d = nc.dram_tensor("wv", (H, H), WQ_T, kind="ExternalInput").ap()
    wo_d = nc.dram_tensor("wo", (H, H), WQ_T, kind="ExternalInput").ap()
    w1_d = nc.dram_tensor("w1", (H, MLPD), W1_T, kind="ExternalInput").ap()
    w2_d = nc.dram_tensor("w2", (MLPD, H), W2_T, kind="ExternalInput").ap()
    bq_d = nc.dram_tensor("bq", (H,), F32, kind="ExternalInput").ap()
    bk_d = nc.dram_tensor("bk", (H,), F32, kind="ExternalInput").ap()
    bv_d = nc.dram_tensor("bv", (H,), BF, kind="ExternalInput").ap()
    bo_d = nc.dram_tensor("bo", (H,), F32, kind="ExternalInput").ap()
    b1_d = nc.dram_tensor("b1", (MLPD,), F32, kind="ExternalInput").ap()
    b2_d = nc.dram_tensor("b2", (H,), F32, kind="ExternalInput").ap()
    ln1w_d = nc.dram_tensor("ln1_w", (H,), BF, kind="ExternalInput").ap()
    ln1b_d = nc.dram_tensor("ln1_b", (H,), BF, kind="ExternalInput").ap()
    ln2w_d = nc.dram_tensor("ln2_w", (H,), BF, kind="ExternalInput").ap()
    ln2b_d = nc.dram_tensor("ln2_b", (H,), BF, kind="ExternalInput").ap()
    out_d = nc.dram_tensor("out_loc", (tq, H), F32, kind="ExternalOutput").ap()

    def bcast(ap1d):
        return bass.AP(
            tensor=ap1d.tensor, offset=ap1d.offset, ap=[[0, P]] + list(ap1d.ap)
        )

    with tile.TileContext(nc) as tc, ExitStack() as top:
        const = top.enter_context(tc.tile_pool(name="const", bufs=1))
        persist = top.enter_context(tc.tile_pool(name="persist", bufs=1))
        # Top-level PSUM pool: 2 banks shared by transposes + proj accums.
        psum = top.enter_context(tc.tile_pool(name="psum", bufs=1, space="PSUM"))
        toks = top.enter_context(tc.tile_pool(name="toks", bufs=4))
        tmps = top.enter_context(tc.tile_pool(name="tmps", bufs=2))

        # ---- constants ----
        ident = const.tile([P, P], BF)
        make_identity(nc, ident)
        eps_t = const.tile([P, 1], F32)
        nc.vector.memset(eps_t, EPS)
        ln1w_bc = const.tile([P, H], BF)
        nc.gpsimd.dma_start(out=ln1w_bc, in_=bcast(ln1w_d))
        ln1b_bc = const.tile([P, H], BF)
        nc.gpsimd.dma_start(out=ln1b_bc, in_=bcast(ln1b_d))
        ln2w_bc = const.tile([P, H], BF)
        nc.gpsimd.dma_start(out=ln2w_bc, in_=bcast(ln2w_d))
        ln2b_bc = const.tile([P, H], BF)
        nc.gpsimd.dma_start(out=ln2b_bc, in_=bcast(ln2b_d))
        bv_row = const.tile([1, H], BF)
        nc.sync.dma_start(out=bv_row, in_=bv_d[None, :])
        ones_row = const.tile([1, P], BF)
        nc.vector.memset(ones_row, 1.0)
        bq_sb = const.tile([P, N_H], F32)
        nc.sync.dma_start(out=bq_sb, in_=bq_d.rearrange("(t p) -> p t", p=P))
        bk_sb = const.tile([P, N_H], F32)
        nc.sync.dma_start(out=bk_sb, in_=bk_d.rearrange("(t p) -> p t", p=P))
        bo_sb = const.tile([P, N_H], F32)
        nc.sync.dma_start(out=bo_sb, in_=bo_d.rearrange("(t p) -> p t", p=P))
        b1_sb = const.tile([P, N_M], F32)
        nc.sync.dma_start(out=b1_sb, in_=b1_d.rearrange("(t p) -> p t", p=P))
        b2_sb = const.tile([P, N_H], F32)
        nc.sync.dma_start(out=b2_sb, in_=b2_d.rearrange("(t p) -> p t", p=P))
        # Weight layout [P, pair, out-tile, slab, P]: the DoubleRow stationary
        # slice [:, g, hot, :, :] is a contiguous 2x128 block (the ISA
        # requires the two slabs adjacent in SBUF).
        wo_sb = const.tile([P, N_H // 2, N_H, 2, P], WQ_T)

        def load_w_pairs(dst, src_d, n_in, n_out):
            for g in range(n_in // 2):
                for s in range(2):
                    i = 2 * g + s
                    nc.sync.dma_start(
                        out=dst[:, g, :, s, :], in_=src_d[i * P : (i + 1) * P, :]
                    )

        ctx_tok = persist.tile([P, n_q, H], BF)  # normalized ctx (token-major)

        def ln_tile(x_ap, w_bc, b_bc, out_bf):
            """LayerNorm of one [P, H] fp32 tile -> bf16 out (token-major).

            Stats on DVE, the normalize pass on ACT (per-partition
            scale/bias), the weight/bias application on DVE in bf16.
            """
            stats = tmps.tile([P, 2, 6], F32, tag="ln_stats", bufs=4)
            for g in range(2):
                nc.vector.bn_stats(out=stats[:, g, :], in_=x_ap[:, g * 384 : (g + 1) * 384])
            mv = tmps.tile([P, 2], F32, tag="ln_mv", bufs=4)
            nc.vector.bn_aggr(out=mv, in_=stats)
            rstd = tmps.tile([P, 1], F32, tag="ln_rstd", bufs=4)
            nc.scalar.activation(out=rstd, in_=mv[:, 1:2], func=AF.Sqrt, bias=eps_t, scale=1.0)
            nc.vector.reciprocal(out=rstd, in_=rstd)
            nmr = tmps.tile([P, 1], F32, tag="ln_nmr", bufs=4)
            nc.vector.scalar_tensor_tensor(
                out=nmr, in0=mv[:, 0:1], scalar=-1.0, in1=rstd,
                op0=AluOpType.mult, op1=AluOpType.mult,
            )
            xh = tmps.tile([P, H], BF, tag="ln_xh", bufs=4)
            nc.scalar.activation(out=xh, in_=x_ap, func=AF.Identity, scale=rstd, bias=nmr)
            nc.vector.tensor_mul(out_bf, xh, w_bc)
            nc.vector.tensor_add(out_bf, out_bf, b_bc)

        def transpose_to(dst_ap, src_ap, rows, cols):
            """dst[cols, rows] = src[rows, cols].T (SBUF->SBUF, evict on DVE).

            The PE transpose stays bf16 (fp8 transpose needs stride-2
            outputs); any fp8 cast happens in the DVE eviction.
            """
            pt = psum.tile([P, P], BF, tag="aux", bufs=2)
            nc.tensor.transpose(pt[0:cols, 0:rows], src_ap, ident[0:rows, 0:rows])
            nc.vector.tensor_copy(out=dst_ap, in_=pt[0:cols, 0:rows])

        # ====== Phase 1-3: LN1, V proj, then attention in two query
        # chunks. Chunk 0 runs with the Q/K projections interleaved; chunk 1
        # runs with chunk 0's downstream work (ctx transpose, out-proj,
        # residual, LN2) woven between head-pairs so the PE chews projection
        # work while ACT grinds exp. ======
        ACH = CH  # attention query-chunk = moving-operand chunk (512)
        n_ac = tq // ACH

        late = top.enter_context(tc.tile_pool(name="late", bufs=1))
        x1_sb = late.tile([P, n_q, H], F32)  # attn-block out (token-major)
        xn2T = late.tile([P, N_H, tq], XN2_T)
        op = top.enter_context(tc.tile_pool(name="oproj", bufs=1))

        with tc.tile_pool(name="qkv_sb", bufs=1) as qkv_sb:
            # Q stored zero-padded per head: head h occupies its 64 rows,
            # the other 64 rows stay zero, so the scores matmul can use the
            # full 128-row kT stationary (FWL) with exact math.
            qT = qkv_sb.tile([P, NH, tq], BF)
            nc.vector.memset(qT, 0.0)
            kT = qkv_sb.tile([P, N_H, tkv], BF)
            # [P, kt-pair, head, slab, 128]: ctx DoubleRow stationary slice
            # [:, ktp, h, :, 0:65] — the ISA wants slab stride 128, so the
            # last dim is padded from HD+1 to 128.
            vone = qkv_sb.tile([P, n_kv // 2, NH, 2, P], EX_T)
            nc.vector.memset(vone[:, :, :, :, HD : HD + 1], 1.0)

            with tc.tile_pool(name="ln_qkv", bufs=1) as lnp, tc.tile_pool(
                name="attn_sb", bufs=1
            ) as asb:
                xnT = lnp.tile([P, N_H, tkv], XN_T)
                wq_sb = lnp.tile([P, N_H // 2, N_H, 2, P], WQ_T)
                wk_sb = lnp.tile([P, N_H // 2, N_H, 2, P], WQ_T)
                wv_sb = lnp.tile([P, N_H, H], WQ_T)
                for i in range(N_H):
                    nc.sync.dma_start(
                        out=wv_sb[:, i, :], in_=wv_d[i * P : (i + 1) * P, :]
                    )

                # LN1 + transpose + V projection, per token tile (keeps PE
                # fed with V matmuls while DVE/ACT chew the next LN). The
                # attention PSUM pool opens only after this loop, so V
                # accumulators and transposes get their own banks here.
                with tc.tile_pool(name="psLN", bufs=1, space="PSUM") as psLN:
                    for t in range(n_kv):
                        x_t = toks.tile([P, H], F32, tag="xtok")
                        nc.sync.dma_start(out=x_t, in_=x_d[t * P : (t + 1) * P, :])
                        xn_bf = tmps.tile([P, H], BF, tag="xn_bf", bufs=4)
                        ln_tile(x_t, ln1w_bc, ln1b_bc, xn_bf)
                        for j in range(N_H):
                            transpose_to(
                                xnT[:, j, t * P : (t + 1) * P],
                                xn_bf[:, j * P : (j + 1) * P], P, P,
                            )
                        for c2 in range(n_vc):
                            pv = psLN.tile([P, VC], F32, tag="pv", bufs=3)
                            # bias row via K=1 ones-matmul, then accumulate
                            nc.tensor.matmul(
                                pv, ones_row[:, 0:P],
                                bv_row[:, c2 * VC : (c2 + 1) * VC],
                                start=True, stop=False,
                            )
                            # V-proj stays non-DoubleRow: its stationary is
                            # the activation tile, which can't be pair-packed
                            # without breaking the QK moving layout.
                            for hit in range(N_H):
                                nc.tensor.matmul(
                                    pv,
                                    xnT[:, hit, t * P : (t + 1) * P],
                                    wv_sb[:, hit, c2 * VC : (c2 + 1) * VC],
                                    start=False, stop=(hit == N_H - 1),
                                )
                            vdst = vone[
                                :, t // 2, c2 * (VC // HD) : (c2 + 1) * (VC // HD),
                                t % 2, 0:HD,
                            ]
                            if FP8_ATTN:
                                nc.scalar.mul(
                                    vdst, pv.rearrange("p (h d) -> p h d", d=HD), DQ,
                                )
                            else:
                                nc.vector.tensor_copy(
                                    out=vdst,
                                    in_=pv.rearrange("p (h d) -> p h d", d=HD),
                                )

                # Q/K/O weights only needed once attention starts; emit
                # their loads after the x/LN traffic so they don't delay it.
                load_w_pairs(wq_sb, wq_d, N_H, N_H)
                load_w_pairs(wk_sb, wk_d, N_H, N_H)
                load_w_pairs(wo_sb, wo_d, N_H, N_H)

                def qk_proj_chunk(w_sb, b_sb, dstT, hot, c, split_q=False):
                        pk = psum.tile([P, CH], F32, tag="aux", bufs=2)
                        if FP8_ATTN:
                            for g in range(N_H // 2):
                                nc.tensor.matmul(
                                    pk,
                                    w_sb[:, g, hot, :, :],
                                    xnT[:, 2 * g : 2 * g + 2, c * CH : (c + 1) * CH],
                                    start=(g == 0), stop=(g == N_H // 2 - 1),
                                    perf_mode=DR,
                                )
                        else:
                            for hit in range(N_H):
                                nc.tensor.matmul(
                                    pk,
                                    w_sb[:, hit // 2, hot, hit % 2, :],
                                    xnT[:, hit, c * CH : (c + 1) * CH],
                                    start=(hit == 0), stop=(hit == N_H - 1),
                                )
                        if split_q:
                            nc.vector.tensor_scalar(
                                out=dstT[0:HD, 2 * hot, c * CH : (c + 1) * CH],
                                in0=pk[0:HD, :], scalar1=ADQ,
                                scalar2=b_sb[:, hot : hot + 1][0:HD],
                                op0=AluOpType.mult, op1=AluOpType.add,
                            )
                            nc.vector.tensor_scalar(
                                out=dstT[HD:P, 2 * hot + 1, c * CH : (c + 1) * CH],
                                in0=pk[HD:P, :], scalar1=ADQ,
                                scalar2=b_sb[:, hot : hot + 1][HD:P],
                                op0=AluOpType.mult, op1=AluOpType.add,
                            )
                        else:
                            nc.vector.tensor_scalar(
                                out=dstT[:, hot, c * CH : (c + 1) * CH],
                                in0=pk, scalar1=ADQ,
                                scalar2=b_sb[:, hot : hot + 1],
                                op0=AluOpType.mult, op1=AluOpType.add,
                            )

                def qk_thunks(ht):
                    ths = []
                    for c in range(n_cq):
                        ths.append(lambda c=c: qk_proj_chunk(
                            wq_sb, bq_sb, qT, ht, c, split_q=True))
                    for c in range(n_ckv):
                        ths.append(lambda c=c: qk_proj_chunk(
                            wk_sb, bk_sb, kT, ht, c))
                    return ths

                psA = []

                def attention_head(h, c):
                    ht = h // 2
                    pctx = psA[0].tile([P, ACH], F32, tag="pctx", bufs=2)
                    for ktp in range(n_kv // 2):
                        # kt-pair scores land in one [P, 2, ACH] PSUM tile so
                        # a single 1024-col exp serves both (ACT has ~100ns
                        # fixed cost per instruction).
                        ps = psA[0].tile([P, 2, ACH], F32, tag="psc", bufs=2)
                        for s in range(2):
                            kt = 2 * ktp + s
                            nc.tensor.matmul(
                                ps[:, s, :],
                                kT[:, ht, kt * P : (kt + 1) * P],
                                qT[:, h, c * ACH : (c + 1) * ACH],
                                start=True, stop=True,
                            )
                        ex2 = asb.tile([P, 2, ACH], EX_T, tag="exp", bufs=4)
                        nc.scalar.activation(
                            out=ex2, in_=ps, func=AF.Exp, scale=0.125,
                        )
                        if FP8_ATTN:
                            nc.tensor.matmul(
                                pctx[0 : HD + 1, :],
                                vone[:, ktp, h, :, 0 : HD + 1],
                                ex2,
                                start=(ktp == 0), stop=(ktp == n_kv // 2 - 1),
                                perf_mode=DR,
                            )
                        else:
                            for s in range(2):
                                nc.tensor.matmul(
                                    pctx[0 : HD + 1, :],
                                    vone[:, ktp, h, s, 0 : HD + 1],
                                    ex2[:, s, :],
                                    start=(ktp == 0 and s == 0),
                                    stop=(ktp == n_kv // 2 - 1 and s == 1),
                                )
                    cd = asb.tile([P, ACH], BF, tag="cd", bufs=3)
                    nc.vector.tensor_copy(out=cd[0 : HD + 1, :], in_=pctx[0 : HD + 1, :])
                    for b4 in range(ACH // P):
                        t_tok = c * (ACH // P) + b4
                        pt = psum.tile([P, P], BF, tag="aux", bufs=2)
                        nc.tensor.transpose(
                            pt[0:P, 0 : HD + 1],
                            cd[0 : HD + 1, b4 * P : (b4 + 1) * P],
                            ident[0 : HD + 1, 0 : HD + 1],
                        )
                        rp = tmps.tile([P, 1], F32, tag="rp", bufs=4)
                        nc.vector.reciprocal(rp, pt[:, HD : HD + 1])
                        nc.vector.tensor_scalar(
                            out=ctx_tok[:, t_tok, h * HD : (h + 1) * HD],
                            in0=pt[:, 0:HD], scalar1=rp, scalar2=CSC,
                            op0=AluOpType.mult, op1=AluOpType.mult,
                        )

                # ---- downstream for one query chunk: ctx transpose,
                # out-proj, residual, LN2 (+ xn2 transpose). Returned as a
                # list of thunks so chunk 0's work can be interleaved
                # between chunk 1's attention head-pairs. ----
                def downstream_pieces(c):
                    ctxT = op.tile([P, N_H, CH], CTX_T, tag="ctxT", bufs=2)
                    uT = op.tile([P, N_H, CH], BF, tag="uT", bufs=2)

                    def ctx_trans(ts_):
                        for t in ts_:
                            for j in range(N_H):
                                transpose_to(
                                    ctxT[:, j, (t - c * n_b) * P : (t - c * n_b + 1) * P],
                                    ctx_tok[:, t, j * P : (j + 1) * P], P, P,
                                )

                    def oproj(hots):
                        for hot in hots:
                            pu = psum.tile([P, CH], F32, tag="aux", bufs=2)
                            if FP8_ATTN:
                                for g in range(N_H // 2):
                                    nc.tensor.matmul(
                                        pu,
                                        wo_sb[:, g, hot, :, :],
                                        ctxT[:, 2 * g : 2 * g + 2, :],
                                        start=(g == 0), stop=(g == N_H // 2 - 1),
                                        perf_mode=DR,
                                    )
                            else:
                                for hit in range(N_H):
                                    nc.tensor.matmul(
                                        pu,
                                        wo_sb[:, hit // 2, hot, hit % 2, :],
                                        ctxT[:, hit, :],
                                        start=(hit == 0), stop=(hit == N_H - 1),
                                    )
                            nc.vector.tensor_scalar(
                                out=uT[:, hot, :],
                                in0=pu, scalar1=ADQ / CSC,
                                scalar2=bo_sb[:, hot : hot + 1],
                                op0=AluOpType.mult, op1=AluOpType.add,
                            )

                    xrs = {}
                    for t in range(c * n_b, (c + 1) * n_b):
                        xr = toks.tile([P, H], F32, tag="xtok")
                        nc.sync.dma_start(out=xr, in_=x_d[t * P : (t + 1) * P, :])
                        xrs[t] = xr

                    def resid_ln2(ts_):
                        for t in ts_:
                            xr = xrs[t]
                            for j in range(N_H):
                                pt = psum.tile([P, P], BF, tag="aux", bufs=2)
                                nc.tensor.transpose(
                                    pt,
                                    uT[:, j, (t - c * n_b) * P : (t - c * n_b + 1) * P],
                                    ident,
                                )
                                nc.vector.tensor_add(
                                    x1_sb[:, t, j * P : (j + 1) * P],
                                    pt,
                                    xr[:, j * P : (j + 1) * P],
                                )
                            xn2_bf = tmps.tile([P, H], BF, tag="xn_bf", bufs=4)
                            ln_tile(x1_sb[:, t, :], ln2w_bc, ln2b_bc, xn2_bf)
                            for j in range(N_H):
                                transpose_to(
                                    xn2T[:, j, t * P : (t + 1) * P],
                                    xn2_bf[:, j * P : (j + 1) * P], P, P,
                                )

                    b0 = c * n_b
                    return [
                        lambda: ctx_trans([b0, b0 + 1]),
                        lambda: ctx_trans([b0 + 2, b0 + 3]),
                        lambda: oproj([0, 1, 2]),
                        lambda: oproj([3, 4, 5]),
                        lambda: resid_ln2([b0, b0 + 1]),
                        lambda: resid_ln2([b0 + 2, b0 + 3]),
                    ]

                # chunk 0: attention with QK projections interleaved per
                # pair; chunk 1: attention with chunk 0's downstream pieces
                # between pairs.
                with tc.tile_pool(name="psA", bufs=1, space="PSUM") as psA_:
                    psA.append(psA_)
                    for ht in range(N_H):
                        for th in qk_thunks(ht):
                            th()
                        attention_head(2 * ht, 0)
                        attention_head(2 * ht + 1, 0)
                    ds0 = downstream_pieces(0)
                    for ht in range(N_H):
                        attention_head(2 * ht, 1)
                        attention_head(2 * ht + 1, 1)
                        ds0[ht]()

        # ========== tail: downstream(c1), then MLP both chunks ==========
        with tc.tile_pool(name="mlp_sb", bufs=1) as mp, tc.tile_pool(
            name="ps6", bufs=1, space="PSUM"
        ) as ps6:
            w1_sb = mp.tile([P, N_H // 2, N_M, 2, P], W1_T)
            w2_sb = mp.tile([P, N_M // 2, N_H, 2, P], W2_T)
            h1c = mp.tile([P, N_M, CH], H1_T)
            y2T = mp.tile([P, N_H, CH], BF)
            # ds(c1) first: its x reloads reach the sync queue ahead of
            # the 7MB of MLP weights; w1 lands under ds(c1), w2 under fc1.
            ds1 = downstream_pieces(1)
            load_w_pairs(w1_sb, w1_d, N_H, N_M)
            for piece in ds1:
                piece()
            load_w_pairs(w2_sb, w2_d, N_M, N_H)

            def mlp_chunk(c):
                for mt in range(N_M):
                    ph = ps6.tile([P, CH], F32, tag="pmm", bufs=4)
                    if FP8_FC1:
                        for g in range(N_H // 2):
                            nc.tensor.matmul(
                                ph,
                                w1_sb[:, g, mt, :, :],
                                xn2T[:, 2 * g : 2 * g + 2, c * CH : (c + 1) * CH],
                                start=(g == 0), stop=(g == N_H // 2 - 1),
                                perf_mode=DR,
                            )
                    else:
                        for hit in range(N_H):
                            nc.tensor.matmul(
                                ph,
                                w1_sb[:, hit // 2, mt, hit % 2, :],
                                xn2T[:, hit, c * CH : (c + 1) * CH],
                                start=(hit == 0), stop=(hit == N_H - 1),
                            )
                    nc.scalar.activation(
                        out=h1c[:, mt, :], in_=ph,
                        func=getattr(AF, mlp_act), bias=b1_sb[:, mt : mt + 1],
                        scale=(DQ if FP8_FC1 else 1.0),
                    )
                for hot in range(N_H):
                    py = ps6.tile([P, CH], F32, tag="pmm", bufs=4)
                    if FP8_FC2:
                        for g in range(N_M // 2):
                            nc.tensor.matmul(
                                py,
                                w2_sb[:, g, hot, :, :],
                                h1c[:, 2 * g : 2 * g + 2, :],
                                start=(g == 0), stop=(g == N_M // 2 - 1),
                                perf_mode=DR,
                            )
                    else:
                        for mt in range(N_M):
                            nc.tensor.matmul(
                                py,
                                w2_sb[:, mt // 2, hot, mt % 2, :],
                                h1c[:, mt, :],
                                start=(mt == 0), stop=(mt == N_M - 1),
                            )
                    nc.vector.tensor_scalar(
                        out=y2T[:, hot, :], in0=py,
                        scalar1=(DQ if FP8_FC2 else 1.0),
                        scalar2=b2_sb[:, hot : hot + 1],
                        op0=AluOpType.mult, op1=AluOpType.add,
                    )
                for b4 in range(n_b):
                    t = c * n_b + b4
                    outt = toks.tile([P, H], F32, tag="xtok")
                    for j in range(N_H):
                        pt = psum.tile([P, P], BF, tag="aux", bufs=2)
                        nc.tensor.transpose(
                            pt, y2T[:, j, b4 * P : (b4 + 1) * P], ident,
                        )
                        nc.vector.tensor_add(
                            outt[:, j * P : (j + 1) * P],
                            pt,
                            x1_sb[:, t, j * P : (j + 1) * P],
                        )
                    nc.sync.dma_start(out=out_d[t * P : (t + 1) * P, :], in_=outt)

            mlp_chunk(0)
            mlp_chunk(1)

    nc.compile()
    _BUILD_CACHE[key] = nc
    return nc


def make_in_maps(inputs, tkv=S):
    """Build the 8 per-core input maps from full inputs."""
    f = np.asarray
    x = f(inputs["x"], dtype=np.float32)
    tq = tkv // 2
    E4NP = ml_dtypes.float8_e4m3

    def wprep(name, fp8):
        w = f(inputs[name], dtype=np.float32)
        if fp8:
            return np.ascontiguousarray((w * WS).astype(E4NP))
        return np.ascontiguousarray(w.astype(ml_dtypes.bfloat16))

    wcast = {
        "wq": wprep("wq", FP8_ATTN),
        "wk": wprep("wk", FP8_ATTN),
        "wv": wprep("wv", FP8_ATTN),
        "wo": wprep("wo", FP8_ATTN),
        "w1": wprep("w1", FP8_FC1),
        "w2": wprep("w2", FP8_FC2),
    }
    fp32v = {
        n: np.ascontiguousarray(f(inputs[n], dtype=np.float32))
        for n in ["bq", "bk", "bo", "b1", "b2"]
    }
    for n in ["ln1_w", "ln1_b", "ln2_w", "ln2_b"]:
        fp32v[n] = np.ascontiguousarray(f(inputs[n]).astype(ml_dtypes.bfloat16))
    bv = f(inputs["bv"], dtype=np.float32)
    if FP8_ATTN:
        bv = bv * WS  # the ones-matmul adds WS*bv into the pre-dequant PSUM
    fp32v["bv"] = np.ascontiguousarray(bv.astype(ml_dtypes.bfloat16))
    in_maps = []
    for c in range(8):
        b, half = c // 2, c % 2
        if half == 0:
            x_loc = x[b, :tkv]
        else:
            x_loc = np.concatenate([x[b, tq:tkv], x[b, :tq]], axis=0)
        m = {"x_loc": np.ascontiguousarray(x_loc)}
        m.update(wcast)
        m.update(fp32v)
        in_maps.append(m)
    return in_maps


def kernel(**inputs):
    nc = build(S)
    in_maps = make_in_maps(inputs, S)
    res = run_bass_kernel_spmd(nc, in_maps, core_ids=list(range(8)))
    tq = S // 2
    out = np.empty((B, S, H), dtype=np.float32)
    for c in range(8):
        b, half = c // 2, c % 2
        out[b, half * tq : (half + 1) * tq] = res.results[c]["out_loc"]
    return out


# revision 30
# speedup vs baseline: 1.1958x; 1.0633x over previous
"""Trainium2 Bass kernel for a dense pre-norm transformer block.

B, S, H, NH, MLP = 4, 2048, 768, 12, 3072 (fp32 I/O).

Sharding: 8 shards = (batch, seq-half). Each core receives its batch's full
2048-token sequence with its own 1024 query tokens permuted to the front
(attention is permutation-invariant over keys), computes K/V for all 2048
tokens, and Q/attention/MLP for its 1024 query tokens. No collectives.

On-chip: activations are kept feature-major [feature-part, token-free] for
matmuls (weights stationary), token-major for LN/softmax-normalize/residual.
Attention computes scoresT = K @ Q^T per head, exponentiates on ACT
(scale=1/8 folded), then multiplies with a stationary [V | ones] so the
softmax denominator accumulates for free in the extra PSUM row; the
normalization happens after a PE transpose back to token-major where the
denominator is a per-partition scalar.

fp8 (e4m3) + DoubleRow perf mode doubles matmul throughput for every
contraction>=256 GEMM: Q/K/V/O projections, the probs@V context matmul
(kt-pair slabs), and both MLP linears. Weights are pre-scaled by WS on the
host so their values sit in e4m3's normal range; the dequant (1/WS) rides
the PSUM-eviction op (fused with bias). Scores stay bf16 (contraction=64
gains nothing from fp8). fp32 accumulation everywhere; fp32 LN/residual.

Schedule: the Q/K projections are interleaved with attention per head-pair so
the tensor engine never idles waiting on ACT exp; PSUM->SBUF copies ride on
DVE/ACT to balance engine load.
"""

import sys

if "/opt/trn_rl_repo" not in sys.path:
    sys.path.insert(0, "/opt/trn_rl_repo")

from contextlib import ExitStack

import ml_dtypes
import numpy as np

import concourse.bacc as bacc
import concourse.bass as bass
import concourse.mybir as mybir
import concourse.tile as tile
from concourse.alu_op_type import AluOpType
from concourse.bass_utils import run_bass_kernel_spmd
from concourse.masks import make_identity

B, S, H, NH, MLPD = 4, 2048, 768, 12, 3072
HD = H // NH  # 64
EPS = 1e-6
P = 128
N_H = H // P  # 6
N_M = MLPD // P  # 24
AF = mybir.ActivationFunctionType
BF = mybir.dt.bfloat16
F32 = mybir.dt.float32
F8 = mybir.dt.float8e4
DR = mybir.MatmulPerfMode.DoubleRow

# fp8 toggles + scales
FP8_ATTN = True
FP8_FC1 = True
FP8_FC2 = False  # fc2 stays bf16: fp8 on both MLP linears costs ~1.8e-2 rel
WS = 1024.0  # host-side fp8 weight pre-scale
DQ = 1.0 / WS
CTX_S = 64.0  # ctx quantization scale (ctx values are small)

_BUILD_CACHE = {}


def build(tkv=S, mlp_act="Gelu"):
    key = (tkv, mlp_act)
    if key in _BUILD_CACHE:
        return _BUILD_CACHE[key]

    tq = tkv // 2
    n_kv = tkv // P  # K/V token tiles
    n_q = tq // P  # query token tiles
    CH = 512 if tq % 512 == 0 else tq  # moving-operand chunk
    n_cq = tq // CH  # query chunks
    n_ckv = tkv // CH  # kv chunks
    n_b = CH // P  # 128-blocks per chunk
    VC = 384  # v-proj output chunk (6 heads)
    n_vc = H // VC  # 2

    WQ_T = F8 if FP8_ATTN else BF
    W1_T = F8 if FP8_FC1 else BF
    W2_T = F8 if FP8_FC2 else BF
    XN_T = F8 if FP8_ATTN else BF
    XN2_T = F8 if FP8_FC1 else BF
    H1_T = F8 if FP8_FC2 else BF
    EX_T = F8 if FP8_ATTN else BF
    CTX_T = F8 if FP8_ATTN else BF
    ADQ = DQ if FP8_ATTN else 1.0
    CSC = CTX_S if FP8_ATTN else 1.0

    nc = bacc.Bacc("TRN2", target_bir_lowering=False, debug=False, num_devices=8)

    x_d = nc.dram_tensor("x_loc", (tkv, H), F32, kind="ExternalInput").ap()
    wq_d = nc.dram_tensor("wq", (H, H), WQ_T, kind="ExternalInput").ap()
    wk_d = nc.dram_tensor("wk", (H, H), WQ_T, kind="ExternalInput").ap()
    wv_d = nc.dram_tensor("wv", (H, H), WQ_T, kind="ExternalInput").ap()
    wo_d = nc.dram_tensor("wo", (H, H), WQ_T, kind="ExternalInput").ap()
    w1_d = nc.dram_tensor("w1", (H, MLPD), W1_T, kind="ExternalInput").ap()
    w2_d = nc.dram_tensor("w2", (MLPD, H), W2_T, kind="ExternalInput").ap()
    bq_d = nc.dram_tensor("bq", (H,), F32, kind="ExternalInput").ap()
    bk_d = nc.dram_tensor("bk", (H,), F32, kind="ExternalInput").ap()
    bv_d = nc.dram_tensor("bv", (H,), BF, kind="ExternalInput").ap()
    bo_d = nc.dram_tensor("bo", (H,), F32, kind="ExternalInput").ap()
    b1_d = nc.dram_tensor("b1", (MLPD,), F32, kind="ExternalInput").ap()
    b2_d = nc.dram_tensor("b2", (H,), F32, kind="ExternalInput").ap()
    ln1w_d = nc.dram_tensor("ln1_w", (H,), BF, kind="ExternalInput").ap()
    ln1b_d = nc.dram_tensor("ln1_b", (H,), BF, kind="ExternalInput").ap()
    ln2w_d = nc.dram_tensor("ln2_w", (H,), BF, kind="ExternalInput").ap()
    ln2b_d = nc.dram_tensor("ln2_b", (H,), BF, kind="ExternalInput").ap()
    out_d = nc.dram_tensor("out_loc", (tq, H), F32, kind="ExternalOutput").ap()

    def bcast(ap1d):
        return bass.AP(
            tensor=ap1d.tensor, offset=ap1d.offset, ap=[[0, P]] + list(ap1d.ap)
        )

    with tile.TileContext(nc) as tc, ExitStack() as top:
        const = top.enter_context(tc.tile_pool(name="const", bufs=1))
        persist = top.enter_context(tc.tile_pool(name="persist", bufs=1))
        # Top-level PSUM pool: 2 banks shared by transposes + proj accums.
        psum = top.enter_context(tc.tile_pool(name="psum", bufs=1, space="PSUM"))
        toks = top.enter_context(tc.tile_pool(name="toks", bufs=5))
        tmps = top.enter_context(tc.tile_pool(name="tmps", bufs=2))

        # ---- constants ----
        ident = const.tile([P, P], BF)
        make_identity(nc, ident)
        eps_t = const.tile([P, 1], F32)
        nc.vector.memset(eps_t, EPS)
        ln1w_bc = const.tile([P, H], BF)
        nc.gpsimd.dma_start(out=ln1w_bc, in_=bcast(ln1w_d))
        ln1b_bc = const.tile([P, H], BF)
        nc.gpsimd.dma_start(out=ln1b_bc, in_=bcast(ln1b_d))
        ln2w_bc = const.tile([P, H], BF)
        nc.gpsimd.dma_start(out=ln2w_bc, in_=bcast(ln2w_d))
        ln2b_bc = const.tile([P, H], BF)
        nc.gpsimd.dma_start(out=ln2b_bc, in_=bcast(ln2b_d))
        bv_row = const.tile([1, H], BF)
        nc.sync.dma_start(out=bv_row, in_=bv_d[None, :])
        ones_row = const.tile([1, P], BF)
        nc.vector.memset(ones_row, 1.0)
        bq_sb = const.tile([P, N_H], F32)
        nc.sync.dma_start(out=bq_sb, in_=bq_d.rearrange("(t p) -> p t", p=P))
        bk_sb = const.tile([P, N_H], F32)
        nc.sync.dma_start(out=bk_sb, in_=bk_d.rearrange("(t p) -> p t", p=P))
        bo_sb = const.tile([P, N_H], F32)
        nc.sync.dma_start(out=bo_sb, in_=bo_d.rearrange("(t p) -> p t", p=P))
        b1_sb = const.tile([P, N_M], F32)
        nc.sync.dma_start(out=b1_sb, in_=b1_d.rearrange("(t p) -> p t", p=P))
        b2_sb = const.tile([P, N_H], F32)
        nc.sync.dma_start(out=b2_sb, in_=b2_d.rearrange("(t p) -> p t", p=P))
        # Weight layout [P, pair, out-tile, slab, P]: the DoubleRow stationary
        # slice [:, g, hot, :, :] is a contiguous 2x128 block (the ISA
        # requires the two slabs adjacent in SBUF).
        wo_sb = const.tile([P, N_H // 2, N_H, 2, P], WQ_T)

        def load_w_pairs(dst, src_d, n_in, n_out):
            for g in range(n_in // 2):
                for s in range(2):
                    i = 2 * g + s
                    nc.sync.dma_start(
                        out=dst[:, g, :, s, :], in_=src_d[i * P : (i + 1) * P, :]
                    )

        ctx_tok = persist.tile([P, n_q, H], BF)  # normalized ctx (token-major)

        def ln_tile(x_ap, w_bc, b_bc, out_bf):
            """LayerNorm of one [P, H] fp32 tile -> bf16 out (token-major).

            Stats on DVE, the normalize pass on ACT (per-partition
            scale/bias), the weight/bias application on DVE in bf16.
            """
            stats = tmps.tile([P, 2, 6], F32, tag="ln_stats", bufs=4)
            for g in range(2):
                nc.vector.bn_stats(out=stats[:, g, :], in_=x_ap[:, g * 384 : (g + 1) * 384])
            mv = tmps.tile([P, 2], F32, tag="ln_mv", bufs=4)
            nc.vector.bn_aggr(out=mv, in_=stats)
            rstd = tmps.tile([P, 1], F32, tag="ln_rstd", bufs=4)
            nc.scalar.activation(out=rstd, in_=mv[:, 1:2], func=AF.Sqrt, bias=eps_t, scale=1.0)
            nc.vector.reciprocal(out=rstd, in_=rstd)
            nmr = tmps.tile([P, 1], F32, tag="ln_nmr", bufs=4)
            nc.vector.scalar_tensor_tensor(
                out=nmr, in0=mv[:, 0:1], scalar=-1.0, in1=rstd,
                op0=AluOpType.mult, op1=AluOpType.mult,
            )
            xh = tmps.tile([P, H], BF, tag="ln_xh", bufs=4)
            nc.scalar.activation(out=xh, in_=x_ap, func=AF.Identity, scale=rstd, bias=nmr)
            nc.vector.tensor_mul(out_bf, xh, w_bc)
            nc.vector.tensor_add(out_bf, out_bf, b_bc)

        def transpose_to(dst_ap, src_ap, rows, cols):
            """dst[cols, rows] = src[rows, cols].T (SBUF->SBUF, evict on DVE).

            The PE transpose stays bf16 (fp8 transpose needs stride-2
            outputs); any fp8 cast happens in the DVE eviction.
            """
            pt = psum.tile([P, P], BF, tag="aux", bufs=2)
            nc.tensor.transpose(pt[0:cols, 0:rows], src_ap, ident[0:rows, 0:rows])
            nc.vector.tensor_copy(out=dst_ap, in_=pt[0:cols, 0:rows])

        # ====== Phase 1-3: LN1, V proj, then attention in two query
        # chunks. Chunk 0 runs with the Q/K projections interleaved; chunk 1
        # runs with chunk 0's downstream work (ctx transpose, out-proj,
        # residual, LN2) woven between head-pairs so the PE chews projection
        # work while ACT grinds exp. ======
        ACH = CH  # attention query-chunk = moving-operand chunk (512)
        n_ac = tq // ACH

        late = top.enter_context(tc.tile_pool(name="late", bufs=1))
        x1_sb = late.tile([P, n_q, H], F32)  # attn-block out (token-major)
        xn2T = late.tile([P, N_H, tq], XN2_T)
        op = top.enter_context(tc.tile_pool(name="oproj", bufs=1))

        with tc.tile_pool(name="qkv_sb", bufs=1) as qkv_sb:
            # Q stored zero-padded per head: head h occupies its 64 rows,
            # the other 64 rows stay zero, so the scores matmul can use the
            # full 128-row kT stationary (FWL) with exact math.
            qT = qkv_sb.tile([P, NH, tq], BF)
            nc.vector.memset(qT, 0.0)
            kT = qkv_sb.tile([P, N_H, tkv], BF)
            # [P, kt-pair, head, slab, 128]: ctx DoubleRow stationary slice
            # [:, ktp, h, :, 0:65] — the ISA wants slab stride 128, so the
            # last dim is padded from HD+1 to 128.
            vone = qkv_sb.tile([P, n_kv // 2, NH, 2, P], EX_T)
            nc.vector.memset(vone[:, :, :, :, HD : HD + 1], 1.0)

            with tc.tile_pool(name="ln_qkv", bufs=1) as lnp, tc.tile_pool(
                name="attn_sb", bufs=1
            ) as asb:
                xnT = lnp.tile([P, N_H, tkv], XN_T)
                wq_sb = lnp.tile([P, N_H // 2, N_H, 2, P], WQ_T)
                wk_sb = lnp.tile([P, N_H // 2, N_H, 2, P], WQ_T)
                wv_sb = lnp.tile([P, N_H, H], WQ_T)
                for i in range(N_H):
                    nc.sync.dma_start(
                        out=wv_sb[:, i, :], in_=wv_d[i * P : (i + 1) * P, :]
                    )

                # LN1 + transpose + V projection, per token tile (keeps PE
                # fed with V matmuls while DVE/ACT chew the next LN). The
                # attention PSUM pool opens only after this loop, so V
                # accumulators and transposes get their own banks here.
                with tc.tile_pool(name="psLN", bufs=1, space="PSUM") as psLN:
                    for t in range(n_kv):
                        x_t = toks.tile([P, H], F32, tag="xtok")
                        nc.sync.dma_start(out=x_t, in_=x_d[t * P : (t + 1) * P, :])
                        xn_bf = tmps.tile([P, H], BF, tag="xn_bf", bufs=4)
                        ln_tile(x_t, ln1w_bc, ln1b_bc, xn_bf)
                        for j in range(N_H):
                            transpose_to(
                                xnT[:, j, t * P : (t + 1) * P],
                                xn_bf[:, j * P : (j + 1) * P], P, P,
                            )
                        for c2 in range(n_vc):
                            pv = psLN.tile([P, VC], F32, tag="pv", bufs=3)
                            # bias row via K=1 ones-matmul, then accumulate
                            nc.tensor.matmul(
                                pv, ones_row[:, 0:P],
                                bv_row[:, c2 * VC : (c2 + 1) * VC],
                                start=True, stop=False,
                            )
                            # V-proj stays non-DoubleRow: its stationary is
                            # the activation tile, which can't be pair-packed
                            # without breaking the QK moving layout.
                            for hit in range(N_H):
                                nc.tensor.matmul(
                                    pv,
                                    xnT[:, hit, t * P : (t + 1) * P],
                                    wv_sb[:, hit, c2 * VC : (c2 + 1) * VC],
                                    start=False, stop=(hit == N_H - 1),
                                )
                            vdst = vone[
                                :, t // 2, c2 * (VC // HD) : (c2 + 1) * (VC // HD),
                                t % 2, 0:HD,
                            ]
                            if FP8_ATTN:
                                nc.scalar.mul(
                                    vdst, pv.rearrange("p (h d) -> p h d", d=HD), DQ,
                                )
                            else:
                                nc.vector.tensor_copy(
                                    out=vdst,
                                    in_=pv.rearrange("p (h d) -> p h d", d=HD),
                                )

                # Q/K/O weights only needed once attention starts; emit
                # their loads after the x/LN traffic so they don't delay it.
                load_w_pairs(wq_sb, wq_d, N_H, N_H)
                load_w_pairs(wk_sb, wk_d, N_H, N_H)
                load_w_pairs(wo_sb, wo_d, N_H, N_H)

                def qk_proj_chunk(w_sb, b_sb, dstT, hot, c, split_q=False):
                        pk = psA[0].tile([P, CH], F32, tag="qkp", bufs=1)
                        if FP8_ATTN:
                            for g in range(N_H // 2):
                                nc.tensor.matmul(
                                    pk,
                                    w_sb[:, g, hot, :, :],
                                    xnT[:, 2 * g : 2 * g + 2, c * CH : (c + 1) * CH],
                                    start=(g == 0), stop=(g == N_H // 2 - 1),
                                    perf_mode=DR,
                                )
                        else:
                            for hit in range(N_H):
                                nc.tensor.matmul(
                                    pk,
                                    w_sb[:, hit // 2, hot, hit % 2, :],
                                    xnT[:, hit, c * CH : (c + 1) * CH],
                                    start=(hit == 0), stop=(hit == N_H - 1),
                                )
                        if split_q:
                            nc.vector.tensor_scalar(
                                out=dstT[0:HD, 2 * hot, c * CH : (c + 1) * CH],
                                in0=pk[0:HD, :], scalar1=ADQ,
                                scalar2=b_sb[:, hot : hot + 1][0:HD],
                                op0=AluOpType.mult, op1=AluOpType.add,
                            )
                            nc.vector.tensor_scalar(
                                out=dstT[HD:P, 2 * hot + 1, c * CH : (c + 1) * CH],
                                in0=pk[HD:P, :], scalar1=ADQ,
                                scalar2=b_sb[:, hot : hot + 1][HD:P],
                                op0=AluOpType.mult, op1=AluOpType.add,
                            )
                        else:
                            nc.vector.tensor_scalar(
                                out=dstT[:, hot, c * CH : (c + 1) * CH],
                                in0=pk, scalar1=ADQ,
                                scalar2=b_sb[:, hot : hot + 1],
                                op0=AluOpType.mult, op1=AluOpType.add,
                            )

                def qk_thunks(ht):
                    ths = []
                    for c in range(n_cq):
                        ths.append(lambda c=c: qk_proj_chunk(
                            wq_sb, bq_sb, qT, ht, c, split_q=True))
                    for c in range(n_ckv):
                        ths.append(lambda c=c: qk_proj_chunk(
                            wk_sb, bk_sb, kT, ht, c))
                    return ths

                psA = []

                def attention_head(h, c, fill=None):
                    ht = h // 2
                    pctx = psA[0].tile([P, ACH], F32, tag="pctx", bufs=1)
                    for ktp in range(n_kv // 2):
                        if fill is not None:
                            th = next(fill, None)
                            if th is not None:
                                th()
                        # kt-pair scores land in one [P, 2, ACH] PSUM tile so
                        # a single 1024-col exp serves both (ACT has ~100ns
                        # fixed cost per instruction).
                        ps = psA[0].tile([P, 2, ACH], F32, tag="psc", bufs=2)
                        for s in range(2):
                            kt = 2 * ktp + s
                            nc.tensor.matmul(
                                ps[:, s, :],
                                kT[:, ht, kt * P : (kt + 1) * P],
                                qT[:, h, c * ACH : (c + 1) * ACH],
                                start=True, stop=True,
                            )
                        ex2 = asb.tile([P, 2, ACH], EX_T, tag="exp", bufs=4)
                        nc.scalar.activation(
                            out=ex2, in_=ps, func=AF.Exp, scale=0.125,
                        )
                        if FP8_ATTN:
                            nc.tensor.matmul(
                                pctx[0 : HD + 1, :],
                                vone[:, ktp, h, :, 0 : HD + 1],
                                ex2,
                                start=(ktp == 0), stop=(ktp == n_kv // 2 - 1),
                                perf_mode=DR,
                            )
                        else:
                            for s in range(2):
                                nc.tensor.matmul(
                                    pctx[0 : HD + 1, :],
                                    vone[:, ktp, h, s, 0 : HD + 1],
                                    ex2[:, s, :],
                                    start=(ktp == 0 and s == 0),
                                    stop=(ktp == n_kv // 2 - 1 and s == 1),
                                )
                    cd = asb.tile([P, ACH], BF, tag="cd", bufs=3)
                    nc.vector.tensor_copy(out=cd[0 : HD + 1, :], in_=pctx[0 : HD + 1, :])
                    for b4 in range(ACH // P):
                        t_tok = c * (ACH // P) + b4
                        pt = psum.tile([P, P], BF, tag="aux", bufs=2)
                        nc.tensor.transpose(
                            pt[0:P, 0 : HD + 1],
                            cd[0 : HD + 1, b4 * P : (b4 + 1) * P],
                            ident[0 : HD + 1, 0 : HD + 1],
                        )
                        rp = tmps.tile([P, 1], F32, tag="rp", bufs=4)
                        nc.vector.reciprocal(rp, pt[:, HD : HD + 1])
                        nc.vector.tensor_scalar(
                            out=ctx_tok[:, t_tok, h * HD : (h + 1) * HD],
                            in0=pt[:, 0:HD], scalar1=rp, scalar2=CSC,
                            op0=AluOpType.mult, op1=AluOpType.mult,
                        )

                # ---- downstream for one query chunk: ctx transpose,
                # out-proj, residual, LN2 (+ xn2 transpose). Returned as a
                # list of thunks so chunk 0's work can be interleaved
                # between chunk 1's attention head-pairs. ----
                def downstream_pieces(c):
                    ctxT = op.tile([P, N_H, CH], CTX_T, tag="ctxT", bufs=2)
                    uT = op.tile([P, N_H, CH], BF, tag="uT", bufs=2)

                    def ctx_trans(ts_):
                        for t in ts_:
                            for j in range(N_H):
                                transpose_to(
                                    ctxT[:, j, (t - c * n_b) * P : (t - c * n_b + 1) * P],
                                    ctx_tok[:, t, j * P : (j + 1) * P], P, P,
                                )

                    def oproj(hots):
                        for hot in hots:
                            pu = psum.tile([P, CH], F32, tag="aux", bufs=2)
                            if FP8_ATTN:
                                for g in range(N_H // 2):
                                    nc.tensor.matmul(
                                        pu,
                                        wo_sb[:, g, hot, :, :],
                                        ctxT[:, 2 * g : 2 * g + 2, :],
                                        start=(g == 0), stop=(g == N_H // 2 - 1),
                                        perf_mode=DR,
                                    )
                            else:
                                for hit in range(N_H):
                                    nc.tensor.matmul(
                                        pu,
                                        wo_sb[:, hit // 2, hot, hit % 2, :],
                                        ctxT[:, hit, :],
                                        start=(hit == 0), stop=(hit == N_H - 1),
                                    )
                            nc.vector.tensor_scalar(
                                out=uT[:, hot, :],
                                in0=pu, scalar1=ADQ / CSC,
                                scalar2=bo_sb[:, hot : hot + 1],
                                op0=AluOpType.mult, op1=AluOpType.add,
                            )

                    xrs = {}
                    for t in range(c * n_b, (c + 1) * n_b):
                        xr = toks.tile([P, H], F32, tag="xtok")
                        nc.sync.dma_start(out=xr, in_=x_d[t * P : (t + 1) * P, :])
                        xrs[t] = xr

                    def resid_ln2(ts_):
                        for t in ts_:
                            xr = xrs[t]
                            for j in range(N_H):
                                pt = psum.tile([P, P], BF, tag="aux", bufs=2)
                                nc.tensor.transpose(
                                    pt,
                                    uT[:, j, (t - c * n_b) * P : (t - c * n_b + 1) * P],
                                    ident,
                                )
                                nc.vector.tensor_add(
                                    x1_sb[:, t, j * P : (j + 1) * P],
                                    pt,
                                    xr[:, j * P : (j + 1) * P],
                                )
                            xn2_bf = tmps.tile([P, H], BF, tag="xn_bf", bufs=4)
                            ln_tile(x1_sb[:, t, :], ln2w_bc, ln2b_bc, xn2_bf)
                            for j in range(N_H):
                                transpose_to(
                                    xn2T[:, j, t * P : (t + 1) * P],
                                    xn2_bf[:, j * P : (j + 1) * P], P, P,
                                )

                    b0 = c * n_b
                    return [
                        lambda: ctx_trans([b0, b0 + 1]),
                        lambda: ctx_trans([b0 + 2, b0 + 3]),
                        lambda: oproj([0, 1, 2]),
                        lambda: oproj([3, 4, 5]),
                        lambda: resid_ln2([b0, b0 + 1]),
                        lambda: resid_ln2([b0 + 2, b0 + 3]),
                    ]

                # chunk 0: attention with QK projections interleaved per
                # pair; chunk 1: attention with chunk 0's downstream pieces
                # between pairs.
                with tc.tile_pool(name="psA", bufs=1, space="PSUM") as psA_:
                    psA.append(psA_)
                    for th in qk_thunks(0):
                        th()
                    for ht in range(N_H):
                        # drip the NEXT pair's QK chunks (PE/DVE only) into
                        # this pair's exp-paced kt loop
                        fill = iter(qk_thunks(ht + 1) if ht + 1 < N_H else [])
                        attention_head(2 * ht, 0, fill)
                        attention_head(2 * ht + 1, 0, fill)
                    ds0 = downstream_pieces(0)
                    for ht in range(N_H):
                        attention_head(2 * ht, 1)
                        attention_head(2 * ht + 1, 1)
                        ds0[ht]()

        # ========== tail: downstream(c1), then MLP both chunks ==========
        with tc.tile_pool(name="mlp_sb", bufs=1) as mp, tc.tile_pool(
            name="ps6", bufs=1, space="PSUM"
        ) as ps6:
            w1_sb = mp.tile([P, N_H // 2, N_M, 2, P], W1_T)
            w2_sb = mp.tile([P, N_M // 2, N_H, 2, P], W2_T)
            h1c = mp.tile([P, N_M, CH], H1_T)
            y2T = mp.tile([P, N_H, CH], BF)
            # ds(c1) first: its x reloads reach the sync queue ahead of
            # the 7MB of MLP weights; w1 lands under ds(c1), w2 under fc1.
            ds1 = downstream_pieces(1)
            load_w_pairs(w1_sb, w1_d, N_H, N_M)
            for piece in ds1:
                piece()
            load_w_pairs(w2_sb, w2_d, N_M, N_H)

            def mlp_chunk(c):
                for mt in range(N_M):
                    ph = ps6.tile([P, CH], F32, tag="pmm", bufs=4)
                    if FP8_FC1:
                        for g in range(N_H // 2):
                            nc.tensor.matmul(
                                ph,
                                w1_sb[:, g, mt, :, :],
                                xn2T[:, 2 * g : 2 * g + 2, c * CH : (c + 1) * CH],
                                start=(g == 0), stop=(g == N_H // 2 - 1),
                                perf_mode=DR,
                            )
                    else:
                        for hit in range(N_H):
                            nc.tensor.matmul(
                                ph,
                                w1_sb[:, hit // 2, mt, hit % 2, :],
                                xn2T[:, hit, c * CH : (c + 1) * CH],
                                start=(hit == 0), stop=(hit == N_H - 1),
                            )
                    nc.scalar.activation(
                        out=h1c[:, mt, :], in_=ph,
                        func=getattr(AF, mlp_act), bias=b1_sb[:, mt : mt + 1],
                        scale=(DQ if FP8_FC1 else 1.0),
                    )
                for hot in range(N_H):
                    py = ps6.tile([P, CH], F32, tag="pmm", bufs=4)
                    if FP8_FC2:
                        for g in range(N_M // 2):
                            nc.tensor.matmul(
                                py,
                                w2_sb[:, g, hot, :, :],
                                h1c[:, 2 * g : 2 * g + 2, :],
                                start=(g == 0), stop=(g == N_M // 2 - 1),
                                perf_mode=DR,
                            )
                    else:
                        for mt in range(N_M):
                            nc.tensor.matmul(
                                py,
                                w2_sb[:, mt // 2, hot, mt % 2, :],
                                h1c[:, mt, :],
                                start=(mt == 0), stop=(mt == N_M - 1),
                            )
                    nc.vector.tensor_scalar(
                        out=y2T[:, hot, :], in0=py,
                        scalar1=(DQ if FP8_FC2 else 1.0),
                        scalar2=b2_sb[:, hot : hot + 1],
                        op0=AluOpType.mult, op1=AluOpType.add,
                    )
                for b4 in range(n_b):
                    t = c * n_b + b4
                    outt = toks.tile([P, H], F32, tag="xtok")
                    for j in range(N_H):
                        pt = psum.tile([P, P], BF, tag="aux", bufs=2)
                        nc.tensor.transpose(
                            pt, y2T[:, j, b4 * P : (b4 + 1) * P], ident,
                        )
                        nc.vector.tensor_add(
                            outt[:, j * P : (j + 1) * P],
                            pt,
                            x1_sb[:, t, j * P : (j + 1) * P],
                        )
                    nc.sync.dma_start(out=out_d[t * P : (t + 1) * P, :], in_=outt)

            mlp_chunk(0)
            mlp_chunk(1)

    nc.compile()
    _BUILD_CACHE[key] = nc
    return nc


def make_in_maps(inputs, tkv=S):
    """Build the 8 per-core input maps from full inputs."""
    f = np.asarray
    x = f(inputs["x"], dtype=np.float32)
    tq = tkv // 2
    E4NP = ml_dtypes.float8_e4m3

    def wprep(name, fp8):
        w = f(inputs[name], dtype=np.float32)
        if fp8:
            return np.ascontiguousarray((w * WS).astype(E4NP))
        return np.ascontiguousarray(w.astype(ml_dtypes.bfloat16))

    wcast = {
        "wq": wprep("wq", FP8_ATTN),
        "wk": wprep("wk", FP8_ATTN),
        "wv": wprep("wv", FP8_ATTN),
        "wo": wprep("wo", FP8_ATTN),
        "w1": wprep("w1", FP8_FC1),
        "w2": wprep("w2", FP8_FC2),
    }
    fp32v = {
        n: np.ascontiguousarray(f(inputs[n], dtype=np.float32))
        for n in ["bq", "bk", "bo", "b1", "b2"]
    }
    for n in ["ln1_w", "ln1_b", "ln2_w", "ln2_b"]:
        fp32v[n] = np.ascontiguousarray(f(inputs[n]).astype(ml_dtypes.bfloat16))
    bv = f(inputs["bv"], dtype=np.float32)
    if FP8_ATTN:
        bv = bv * WS  # the ones-matmul adds WS*bv into the pre-dequant PSUM
    fp32v["bv"] = np.ascontiguousarray(bv.astype(ml_dtypes.bfloat16))
    in_maps = []
    for c in range(8):
        b, half = c // 2, c % 2
        if half == 0:
            x_loc = x[b, :tkv]
        else:
            x_loc = np.concatenate([x[b, tq:tkv], x[b, :tq]], axis=0)
        m = {"x_loc": np.ascontiguousarray(x_loc)}
        m.update(wcast)
        m.update(fp32v)
        in_maps.append(m)
    return in_maps


def kernel(**inputs):
    nc = build(S)
    in_maps = make_in_maps(inputs, S)
    res = run_bass_kernel_spmd(nc, in_maps, core_ids=list(range(8)))
    tq = S // 2
    out = np.empty((B, S, H), dtype=np.float32)
    for c in range(8):
        b, half = c // 2, c % 2
        out[b, half * tq : (half + 1) * tq] = res.results[c]["out_loc"]
    return out
